# revision 21
# baseline (speedup 1.0000x reference)
"""EpisodicSlotWriter Trainium2 kernel.

Math (forward values only — the straight-through trick makes write_w equal the
hard one-hot of argmax(sim)):
  sim[b,k]   = dot(epi_keys[b,k], wk_n[b]) / (||epi_keys[b,k]|| + EPS)
  top[b]     = argmax_k sim[b,k];  best[b] = max_k sim[b,k]
  keys_new   = normalize(epi_keys) rows, except row top[b] which is the blended
               row (1-r)*epi_keys + r*write_key, normalized; r = 0.5*clip(ws,0,1)
  vals_new   = epi_vals, except row top[b] blended the same way
  age_new    = (epi_age+1) zeroed at top[b]
  str_new    = clip(0.995*str [+ ws*(1-0.995*str) at top[b]], 0.001, 1.0)

Device (8 NeuronCores, batch-parallel, 8 batches/core) streams the heavy
tensors: normalizes every epi_keys row + computes sim (the full 256MB keys
in/out), and copies epi_vals to vals_new (256MB in/out) via DRAM->DRAM DMA.
Host does the O(B*D) fix-up of the single written row per batch plus the tiny
(B,K) age/strength maps.
"""

import os
from contextlib import ExitStack

import numpy as np

EPS = 1e-6
B, K, D = 64, 2048, 512
M = 8            # cores
BPC = B // M     # batches per core
P = 128          # partitions
TPB = K // P     # k-tiles per batch

_PROGRAM = None
LAST_RESULT = None


def _emit_body(nc, tc, mybir, f32, keys_in, vals_in, wsrc,
               keys_out, vals_out, sim_out, variant="v1"):
    with ExitStack() as ctx:
        kpool = ctx.enter_context(tc.tile_pool(name="kpool", bufs=24))
        spool = ctx.enter_context(tc.tile_pool(name="spool", bufs=3))
        wpool = ctx.enter_context(tc.tile_pool(name="wpool", bufs=2))
        cpool = ctx.enter_context(tc.tile_pool(name="cpool", bufs=3))
        simpool = ctx.enter_context(tc.tile_pool(name="simpool", bufs=1))
        if variant == "v2b":
            onepool = ctx.enter_context(tc.tile_pool(name="onepool", bufs=1))
            ppool = ctx.enter_context(
                tc.tile_pool(name="ppool", bufs=2, space="PSUM")
            )


        # Pass-through copy of epi_vals, DRAM->DRAM, in 4MB chunks.
        n_chunks = 8
        rpc = BPC * K // n_chunks
        for c in range(n_chunks):
            nc.sync.dma_start(
                out=vals_out[c * rpc : (c + 1) * rpc, :],
                in_=vals_in[c * rpc : (c + 1) * rpc, :],
            )

        sim_sb = simpool.tile([P, BPC * TPB], f32, name="sim_sb", tag="sim_sb")

        if variant == "v2b":
            ones = onepool.tile([1, P], f32, name="ones", tag="ones")
            nc.vector.memset(ones[:], 1.0)

        for b in range(BPC):
            if variant == "v2b":
                w1 = wpool.tile([1, D], f32, name="w1", tag="w1")
                nc.sync.dma_start(w1[:], wsrc[b : b + 1, :])
                wt = ppool.tile([P, D], f32, name="wp", tag="wp")
                nc.tensor.matmul(wt[:], lhsT=ones[:], rhs=w1[:],
                                 start=True, stop=True)
            else:
                wt = wpool.tile([P, D], f32, name="wt", tag="wt")
                nc.sync.dma_start(wt[:], wsrc[b])
            n2 = cpool.tile([P, TPB], f32, name="n2", tag="n2")
            dot = cpool.tile([P, TPB], f32, name="dot", tag="dot")
            kts = []
            for t in range(TPB):
                r0 = b * K + t * P
                kt = kpool.tile([P, D], f32, name="kt", tag="kt")
                nc.sync.dma_start(kt[:], keys_in[r0 : r0 + P, :])
                # ACT: norm2 per row (square + accumulate along free axis)
                sq = spool.tile([P, D], f32, name="sq", tag="sq")
                nc.scalar.activation(
                    sq[:],
                    kt[:],
                    mybir.ActivationFunctionType.Square,
                    accum_out=n2[:, t : t + 1],
                )
                # DVE: dot per row with the broadcast write-key
                # (tensor_tensor_reduce crashes the runtime; this
                # InstTensorScalarPtr form computes kt*wt with accum-sum)
                prod = spool.tile([P, D], f32, name="prod", tag="prod")
                nc.vector.scalar_tensor_tensor(
                    out=prod[:],
                    in0=kt[:],
                    scalar=1.0,
                    in1=wt[:],
                    op0=mybir.AluOpType.mult,
                    op1=mybir.AluOpType.mult,
                    accum_out=dot[:, t : t + 1],
                )
                kts.append((kt, r0))

            # inv = 1 / (sqrt(n2) + EPS), batched over the 16 tile columns
            sg = cpool.tile([P, TPB], f32, name="sg", tag="sg")
            nc.scalar.activation(sg[:], n2[:], mybir.ActivationFunctionType.Sqrt)
            nc.vector.tensor_scalar_add(sg[:], sg[:], EPS)
            inv = cpool.tile([P, TPB], f32, name="inv", tag="inv")
            nc.vector.reciprocal(inv[:], sg[:])
            nc.vector.tensor_mul(
                sim_sb[:, b * TPB : (b + 1) * TPB], dot[:], inv[:]
            )

            for t, (kt, r0) in enumerate(kts):
                nc.vector.tensor_scalar_mul(kt[:], kt[:], inv[:, t : t + 1])
                nc.sync.dma_start(keys_out[r0 : r0 + P, :], kt[:])

        nc.sync.dma_start(sim_out[:], sim_sb[:])


def _build_program(reps=1, variant="v1"):
    import concourse.bacc as bacc
    import concourse.mybir as mybir
    import concourse.tile as tile

    f32 = mybir.dt.float32
    nc = bacc.Bacc(
        "TRN2",
        target_bir_lowering=False,
        debug=False,
        num_devices=M,
    )

    keys_in = nc.dram_tensor("keys_in", [BPC * K, D], f32, kind="ExternalInput").ap()
    vals_in = nc.dram_tensor("vals_in", [BPC * K, D], f32, kind="ExternalInput").ap()
    if variant == "v2b":
        wsrc = nc.dram_tensor("wkn", [BPC, D], f32, kind="ExternalInput").ap()
    else:
        wsrc = nc.dram_tensor("wbc", [BPC, P, D], f32, kind="ExternalInput").ap()
    keys_out = nc.dram_tensor("keys_out", [BPC * K, D], f32, kind="ExternalOutput").ap()
    vals_out = nc.dram_tensor("vals_out", [BPC * K, D], f32, kind="ExternalOutput").ap()
    sim_out = nc.dram_tensor("sim_out", [P, BPC * TPB], f32, kind="ExternalOutput").ap()

    with tile.TileContext(nc) as tc:
        for _rep in range(reps):
            _emit_body(nc, tc, mybir, f32, keys_in, vals_in, wsrc,
                       keys_out, vals_out, sim_out, variant=variant)

    nc.compile()
    return nc


def _get_program():
    global _PROGRAM
    if _PROGRAM is None:
        _PROGRAM = _build_program()
    return _PROGRAM


def kernel(**inputs):
    global LAST_RESULT
    from concourse.bass_utils import run_bass_kernel_spmd

    wk = np.asarray(inputs["write_key"], dtype=np.float32)
    wv = np.asarray(inputs["write_val"], dtype=np.float32)
    ws_raw = np.asarray(inputs["write_strength"], dtype=np.float32)
    ek = np.ascontiguousarray(np.asarray(inputs["epi_keys"], dtype=np.float32))
    ev = np.ascontiguousarray(np.asarray(inputs["epi_vals"], dtype=np.float32))
    ea = np.asarray(inputs["epi_age"], dtype=np.float32)
    es = np.asarray(inputs["epi_strength"], dtype=np.float32)

    nc = _get_program()

    wk_n = wk / (np.linalg.norm(wk, axis=-1, keepdims=True) + np.float32(EPS))
    in_maps = []
    for c in range(M):
        sl = slice(c * BPC, (c + 1) * BPC)
        in_maps.append(
            {
                "keys_in": ek[sl].reshape(BPC * K, D),
                "vals_in": ev[sl].reshape(BPC * K, D),
                "wbc": np.ascontiguousarray(
                    np.broadcast_to(wk_n[sl][:, None, :], (BPC, P, D))
                ),
            }
        )

    trace = bool(int(os.environ.get("KERNEL_TRACE", "0")))
    res = run_bass_kernel_spmd(nc, in_maps, list(range(M)), trace=trace)
    LAST_RESULT = res

    keys_new = np.empty((B, K, D), dtype=np.float32)
    vals_new = np.empty((B, K, D), dtype=np.float32)
    sim = np.empty((B, K), dtype=np.float32)
    for c in range(M):
        sl = slice(c * BPC, (c + 1) * BPC)
        keys_new[sl] = res.results[c]["keys_out"].reshape(BPC, K, D)
        vals_new[sl] = res.results[c]["vals_out"].reshape(BPC, K, D)
        # sim_out is [p, b*TPB + t] with k = t*P + p
        sim_raw = res.results[c]["sim_out"].reshape(P, BPC, TPB)
        sim[sl] = sim_raw.transpose(1, 2, 0).reshape(BPC, K)

    top = sim.argmax(axis=1)
    best_sim = sim.max(axis=1)
    slot_idx = top.astype(np.int32)

    ws = np.clip(ws_raw, 0.0, 1.0)
    r = ws * np.float32(0.5)  # (B,)
    ar = np.arange(B)

    # fix the written row of keys (blend then normalize)
    kb = (1.0 - r)[:, None] * ek[ar, top] + r[:, None] * wk
    kbn = kb / (np.linalg.norm(kb, axis=-1, keepdims=True) + np.float32(EPS))
    keys_new[ar, top] = kbn

    # fix the written row of vals
    vals_new[ar, top] = (1.0 - r)[:, None] * ev[ar, top] + r[:, None] * wv

    age_new = ea + np.float32(1.0)
    age_new[ar, top] = 0.0

    s0 = es * np.float32(0.995)
    str_new = np.clip(s0, 0.001, 1.0).astype(np.float32)
    str_new[ar, top] = np.clip(
        s0[ar, top] + ws * (np.float32(1.0) - s0[ar, top]), 0.001, 1.0
    )

    return keys_new, vals_new, age_new, str_new, slot_idx, best_sim


# revision 22
# speedup vs baseline: 1.1593x; 1.1593x over previous
"""EpisodicSlotWriter Trainium2 kernel.

Math (forward values only — the straight-through trick makes write_w equal the
hard one-hot of argmax(sim)):
  sim[b,k]   = dot(epi_keys[b,k], wk_n[b]) / (||epi_keys[b,k]|| + EPS)
  top[b]     = argmax_k sim[b,k];  best[b] = max_k sim[b,k]
  keys_new   = normalize(epi_keys) rows, except row top[b] which is the blended
               row (1-r)*epi_keys + r*write_key, normalized; r = 0.5*clip(ws,0,1)
  vals_new   = epi_vals, except row top[b] blended the same way
  age_new    = (epi_age+1) zeroed at top[b]
  str_new    = clip(0.995*str [+ ws*(1-0.995*str) at top[b]], 0.001, 1.0)

Device (8 NeuronCores, batch-parallel, 8 batches/core) streams the heavy
tensors: normalizes every epi_keys row + computes sim (the full 256MB keys
in/out), and copies epi_vals to vals_new (256MB in/out) via DRAM->DRAM DMA.
Host does the O(B*D) fix-up of the single written row per batch plus the tiny
(B,K) age/strength maps.
"""

import os
from contextlib import ExitStack

import numpy as np

EPS = 1e-6
B, K, D = 64, 2048, 512
M = 8            # cores
BPC = B // M     # batches per core
P = 128          # partitions
TPB = K // P     # k-tiles per batch

_PROGRAM = None
LAST_RESULT = None


def _emit_body(nc, tc, mybir, f32, keys_in, vals_in, wsrc,
               keys_out, vals_out, sim_out, variant="v1"):
    with ExitStack() as ctx:
        kpool = ctx.enter_context(tc.tile_pool(name="kpool", bufs=24))
        spool = ctx.enter_context(tc.tile_pool(name="spool", bufs=3))
        wpool = ctx.enter_context(tc.tile_pool(name="wpool", bufs=2))
        cpool = ctx.enter_context(tc.tile_pool(name="cpool", bufs=3))
        simpool = ctx.enter_context(tc.tile_pool(name="simpool", bufs=1))
        if variant == "v2b":
            onepool = ctx.enter_context(tc.tile_pool(name="onepool", bufs=1))
            ppool = ctx.enter_context(
                tc.tile_pool(name="ppool", bufs=2, space="PSUM")
            )


        # Pass-through copy of epi_vals, DRAM->DRAM, in 4MB chunks.
        n_chunks = 8
        rpc = BPC * K // n_chunks
        for c in range(n_chunks):
            nc.sync.dma_start(
                out=vals_out[c * rpc : (c + 1) * rpc, :],
                in_=vals_in[c * rpc : (c + 1) * rpc, :],
            )

        sim_sb = simpool.tile([P, BPC * TPB], f32, name="sim_sb", tag="sim_sb")

        if variant == "v2b":
            ones = onepool.tile([1, P], f32, name="ones", tag="ones")
            nc.vector.memset(ones[:], 1.0)

        for b in range(BPC):
            if variant == "v2b":
                w1 = wpool.tile([1, D], f32, name="w1", tag="w1")
                nc.sync.dma_start(w1[:], wsrc[b : b + 1, :])
                wt = ppool.tile([P, D], f32, name="wp", tag="wp")
                nc.tensor.matmul(wt[:], lhsT=ones[:], rhs=w1[:],
                                 start=True, stop=True)
            else:
                wt = wpool.tile([P, D], f32, name="wt", tag="wt")
                nc.sync.dma_start(wt[:], wsrc[b])
            n2 = cpool.tile([P, TPB], f32, name="n2", tag="n2")
            dot = cpool.tile([P, TPB], f32, name="dot", tag="dot")
            kts = []
            for t in range(TPB):
                r0 = b * K + t * P
                kt = kpool.tile([P, D], f32, name="kt", tag="kt")
                nc.sync.dma_start(kt[:], keys_in[r0 : r0 + P, :])
                # ACT: norm2 per row (square + accumulate along free axis)
                sq = spool.tile([P, D], f32, name="sq", tag="sq")
                nc.scalar.activation(
                    sq[:],
                    kt[:],
                    mybir.ActivationFunctionType.Square,
                    accum_out=n2[:, t : t + 1],
                )
                # DVE: dot per row with the broadcast write-key
                # (tensor_tensor_reduce crashes the runtime; this
                # InstTensorScalarPtr form computes kt*wt with accum-sum)
                prod = spool.tile([P, D], f32, name="prod", tag="prod")
                nc.vector.scalar_tensor_tensor(
                    out=prod[:],
                    in0=kt[:],
                    scalar=1.0,
                    in1=wt[:],
                    op0=mybir.AluOpType.mult,
                    op1=mybir.AluOpType.mult,
                    accum_out=dot[:, t : t + 1],
                )
                kts.append((kt, r0))

            # inv = 1 / (sqrt(n2) + EPS), batched over the 16 tile columns
            sg = cpool.tile([P, TPB], f32, name="sg", tag="sg")
            nc.scalar.activation(sg[:], n2[:], mybir.ActivationFunctionType.Sqrt)
            nc.vector.tensor_scalar_add(sg[:], sg[:], EPS)
            inv = cpool.tile([P, TPB], f32, name="inv", tag="inv")
            nc.vector.reciprocal(inv[:], sg[:])
            nc.vector.tensor_mul(
                sim_sb[:, b * TPB : (b + 1) * TPB], dot[:], inv[:]
            )

            for t, (kt, r0) in enumerate(kts):
                nc.vector.tensor_scalar_mul(kt[:], kt[:], inv[:, t : t + 1])
                nc.sync.dma_start(keys_out[r0 : r0 + P, :], kt[:])

        nc.sync.dma_start(sim_out[:], sim_sb[:])


def _build_program(reps=1, variant="v1"):
    import concourse.bacc as bacc
    import concourse.mybir as mybir
    import concourse.tile as tile

    f32 = mybir.dt.float32
    nc = bacc.Bacc(
        "TRN2",
        target_bir_lowering=False,
        debug=False,
        num_devices=M,
    )

    keys_in = nc.dram_tensor("keys_in", [BPC * K, D], f32, kind="ExternalInput").ap()
    vals_in = nc.dram_tensor("vals_in", [BPC * K, D], f32, kind="ExternalInput").ap()
    if variant == "v2b":
        wsrc = nc.dram_tensor("wkn", [BPC, D], f32, kind="ExternalInput").ap()
    else:
        wsrc = nc.dram_tensor("wbc", [BPC, P, D], f32, kind="ExternalInput").ap()
    keys_out = nc.dram_tensor("keys_out", [BPC * K, D], f32, kind="ExternalOutput").ap()
    vals_out = nc.dram_tensor("vals_out", [BPC * K, D], f32, kind="ExternalOutput").ap()
    sim_out = nc.dram_tensor("sim_out", [P, BPC * TPB], f32, kind="ExternalOutput").ap()

    with tile.TileContext(nc) as tc:
        for _rep in range(reps):
            _emit_body(nc, tc, mybir, f32, keys_in, vals_in, wsrc,
                       keys_out, vals_out, sim_out, variant=variant)

    nc.compile()
    return nc


def _get_program():
    global _PROGRAM
    if _PROGRAM is None:
        _PROGRAM = _build_program()
    return _PROGRAM


def kernel(**inputs):
    global LAST_RESULT
    from concourse.bass_utils import run_bass_kernel_spmd

    wk = np.asarray(inputs["write_key"], dtype=np.float32)
    wv = np.asarray(inputs["write_val"], dtype=np.float32)
    ws_raw = np.asarray(inputs["write_strength"], dtype=np.float32)
    ek = np.ascontiguousarray(np.asarray(inputs["epi_keys"], dtype=np.float32))
    ev = np.ascontiguousarray(np.asarray(inputs["epi_vals"], dtype=np.float32))
    ea = np.asarray(inputs["epi_age"], dtype=np.float32)
    es = np.asarray(inputs["epi_strength"], dtype=np.float32)

    nc = _get_program()

    wk_n = wk / (np.linalg.norm(wk, axis=-1, keepdims=True) + np.float32(EPS))
    in_maps = []
    for c in range(M):
        sl = slice(c * BPC, (c + 1) * BPC)
        in_maps.append(
            {
                "keys_in": ek[sl].reshape(BPC * K, D),
                "vals_in": ev[sl].reshape(BPC * K, D),
                "wbc": np.ascontiguousarray(
                    np.broadcast_to(wk_n[sl][:, None, :], (BPC, P, D))
                ),
            }
        )

    trace = bool(int(os.environ.get("KERNEL_TRACE", "0")))
    try:
        res = run_bass_kernel_spmd(nc, in_maps, list(range(M)), trace=trace)
    except Exception:
        # one retry for transient runtime errors (e.g. a device left wedged
        # by a previous tenant's crashed session)
        import time as _time

        _time.sleep(5)
        res = run_bass_kernel_spmd(nc, in_maps, list(range(M)), trace=trace)
    LAST_RESULT = res

    keys_new = np.empty((B, K, D), dtype=np.float32)
    vals_new = np.empty((B, K, D), dtype=np.float32)
    sim = np.empty((B, K), dtype=np.float32)
    for c in range(M):
        sl = slice(c * BPC, (c + 1) * BPC)
        keys_new[sl] = res.results[c]["keys_out"].reshape(BPC, K, D)
        vals_new[sl] = res.results[c]["vals_out"].reshape(BPC, K, D)
        # sim_out is [p, b*TPB + t] with k = t*P + p
        sim_raw = res.results[c]["sim_out"].reshape(P, BPC, TPB)
        sim[sl] = sim_raw.transpose(1, 2, 0).reshape(BPC, K)

    top = sim.argmax(axis=1)
    best_sim = sim.max(axis=1)
    slot_idx = top.astype(np.int32)

    ws = np.clip(ws_raw, 0.0, 1.0)
    r = ws * np.float32(0.5)  # (B,)
    ar = np.arange(B)

    # fix the written row of keys (blend then normalize)
    kb = (1.0 - r)[:, None] * ek[ar, top] + r[:, None] * wk
    kbn = kb / (np.linalg.norm(kb, axis=-1, keepdims=True) + np.float32(EPS))
    keys_new[ar, top] = kbn

    # fix the written row of vals
    vals_new[ar, top] = (1.0 - r)[:, None] * ev[ar, top] + r[:, None] * wv

    age_new = ea + np.float32(1.0)
    age_new[ar, top] = 0.0

    s0 = es * np.float32(0.995)
    str_new = np.clip(s0, 0.001, 1.0).astype(np.float32)
    str_new[ar, top] = np.clip(
        s0[ar, top] + ws * (np.float32(1.0) - s0[ar, top]), 0.001, 1.0
    )

    return keys_new, vals_new, age_new, str_new, slot_idx, best_sim


# revision 32
# speedup vs baseline: 1.4405x; 1.2425x over previous
"""EpisodicSlotWriter Trainium2 kernel.

Math (forward values only — the straight-through trick makes write_w equal the
hard one-hot of argmax(sim)):
  sim[b,k]   = dot(epi_keys[b,k], wk_n[b]) / (||epi_keys[b,k]|| + EPS)
  top[b]     = argmax_k sim[b,k];  best[b] = max_k sim[b,k]
  keys_new   = normalize(epi_keys) rows, except row top[b] which is the blended
               row (1-r)*epi_keys + r*write_key, normalized; r = 0.5*clip(ws,0,1)
  vals_new   = epi_vals, except row top[b] blended the same way
  age_new    = (epi_age+1) zeroed at top[b]
  str_new    = clip(0.995*str [+ ws*(1-0.995*str) at top[b]], 0.001, 1.0)

Device (8 NeuronCores, batch-parallel, 8 batches/core) streams the heavy
tensors: normalizes every epi_keys row + computes sim (the full 256MB keys
in/out), and copies epi_vals to vals_new (256MB in/out) via DRAM->DRAM DMA.
Host does the O(B*D) fix-up of the single written row per batch plus the tiny
(B,K) age/strength maps.
"""

import os
from contextlib import ExitStack

import numpy as np

EPS = 1e-6
B, K, D = 64, 2048, 512
M = 8            # cores
BPC = B // M     # batches per core
P = 128          # partitions
TPB = K // P     # k-tiles per batch

_PROGRAM = None
LAST_RESULT = None


def _emit_body(nc, tc, mybir, f32, keys_in, vals_in, wsrc,
               keys_out, vals_out, sim_out, variant="v1"):
    if variant.startswith("v4") or variant.startswith("v8"):
        _emit_body_fused(nc, tc, mybir, f32, keys_in, vals_in, wsrc,
                         keys_out, vals_out, sim_out, variant)
        return
    with ExitStack() as ctx:
        kpool = ctx.enter_context(tc.tile_pool(name="kpool", bufs=24))
        spool = ctx.enter_context(tc.tile_pool(name="spool", bufs=3))
        wpool = ctx.enter_context(tc.tile_pool(name="wpool", bufs=2))
        cpool = ctx.enter_context(tc.tile_pool(name="cpool", bufs=3))
        simpool = ctx.enter_context(tc.tile_pool(name="simpool", bufs=1))
        if variant == "v2b":
            onepool = ctx.enter_context(tc.tile_pool(name="onepool", bufs=1))
            ppool = ctx.enter_context(
                tc.tile_pool(name="ppool", bufs=2, space="PSUM")
            )


        # Pass-through copy of epi_vals, DRAM->DRAM, in 4MB chunks.
        n_chunks = 8
        rpc = BPC * K // n_chunks
        for c in range(n_chunks):
            nc.sync.dma_start(
                out=vals_out[c * rpc : (c + 1) * rpc, :],
                in_=vals_in[c * rpc : (c + 1) * rpc, :],
            )

        sim_sb = simpool.tile([P, BPC * TPB], f32, name="sim_sb", tag="sim_sb")

        if variant == "v2b":
            ones = onepool.tile([1, P], f32, name="ones", tag="ones")
            nc.vector.memset(ones[:], 1.0)

        for b in range(BPC):
            if variant == "v2b":
                w1 = wpool.tile([1, D], f32, name="w1", tag="w1")
                nc.sync.dma_start(w1[:], wsrc[b : b + 1, :])
                wt = ppool.tile([P, D], f32, name="wp", tag="wp")
                nc.tensor.matmul(wt[:], lhsT=ones[:], rhs=w1[:],
                                 start=True, stop=True)
            else:
                wt = wpool.tile([P, D], f32, name="wt", tag="wt")
                nc.sync.dma_start(wt[:], wsrc[b])
            n2 = cpool.tile([P, TPB], f32, name="n2", tag="n2")
            dot = cpool.tile([P, TPB], f32, name="dot", tag="dot")
            kts = []
            for t in range(TPB):
                r0 = b * K + t * P
                kt = kpool.tile([P, D], f32, name="kt", tag="kt")
                nc.sync.dma_start(kt[:], keys_in[r0 : r0 + P, :])
                # ACT: norm2 per row (square + accumulate along free axis)
                sq = spool.tile([P, D], f32, name="sq", tag="sq")
                nc.scalar.activation(
                    sq[:],
                    kt[:],
                    mybir.ActivationFunctionType.Square,
                    accum_out=n2[:, t : t + 1],
                )
                # DVE: dot per row with the broadcast write-key
                # (tensor_tensor_reduce crashes the runtime; this
                # InstTensorScalarPtr form computes kt*wt with accum-sum)
                prod = spool.tile([P, D], f32, name="prod", tag="prod")
                nc.vector.scalar_tensor_tensor(
                    out=prod[:],
                    in0=kt[:],
                    scalar=1.0,
                    in1=wt[:],
                    op0=mybir.AluOpType.mult,
                    op1=mybir.AluOpType.mult,
                    accum_out=dot[:, t : t + 1],
                )
                kts.append((kt, r0))

            # inv = 1 / (sqrt(n2) + EPS), batched over the 16 tile columns
            sg = cpool.tile([P, TPB], f32, name="sg", tag="sg")
            nc.scalar.activation(sg[:], n2[:], mybir.ActivationFunctionType.Sqrt)
            nc.vector.tensor_scalar_add(sg[:], sg[:], EPS)
            inv = cpool.tile([P, TPB], f32, name="inv", tag="inv")
            nc.vector.reciprocal(inv[:], sg[:])
            nc.vector.tensor_mul(
                sim_sb[:, b * TPB : (b + 1) * TPB], dot[:], inv[:]
            )

            for t, (kt, r0) in enumerate(kts):
                nc.vector.tensor_scalar_mul(kt[:], kt[:], inv[:, t : t + 1])
                nc.sync.dma_start(keys_out[r0 : r0 + P, :], kt[:])

        nc.sync.dma_start(sim_out[:], sim_sb[:])


def _emit_body_fused(nc, tc, mybir, f32, keys_in, vals_in, wbc,
                     keys_out, vals_out, sim_out, variant):
    """Like the v1 body, but keys move in [128, NW*512] tiles: one 1MB DMA
    per NW k-chunks, compute on 512-wide slices. 'v8' additionally issues
    keys stores on the ACT HWDGE ring to split descriptor generation."""
    NW = 8 if "w8" in variant else 4
    spread_vals = "s" in variant[2:].replace("w8", "").replace("b", "")
    store_engine = nc.scalar if variant.startswith("v8") else nc.sync
    nbufs = (8 if NW == 4 else 4)
    if "b" in variant[2:]:
        nbufs = int(variant.split("b")[1])
    with ExitStack() as ctx:
        kpool = ctx.enter_context(tc.tile_pool(name="kpool", bufs=nbufs))
        spool = ctx.enter_context(tc.tile_pool(name="spool", bufs=3))
        wpool = ctx.enter_context(tc.tile_pool(name="wpool", bufs=2))
        cpool = ctx.enter_context(tc.tile_pool(name="cpool", bufs=3))
        simpool = ctx.enter_context(tc.tile_pool(name="simpool", bufs=1))

        n_chunks = 8
        rpc = BPC * K // n_chunks
        if not spread_vals:
            for c in range(n_chunks):
                nc.sync.dma_start(
                    out=vals_out[c * rpc : (c + 1) * rpc, :],
                    in_=vals_in[c * rpc : (c + 1) * rpc, :],
                )

        sim_sb = simpool.tile([P, BPC * TPB], f32, name="sim_sb", tag="sim_sb")

        for b in range(BPC):
            if spread_vals:
                nc.sync.dma_start(
                    out=vals_out[b * rpc : (b + 1) * rpc, :],
                    in_=vals_in[b * rpc : (b + 1) * rpc, :],
                )
            wt = wpool.tile([P, D], f32, name="wt", tag="wt")
            nc.sync.dma_start(wt[:], wbc[b])
            n2 = cpool.tile([P, TPB], f32, name="n2", tag="n2")
            dot = cpool.tile([P, TPB], f32, name="dot", tag="dot")
            groups = []
            for g in range(TPB // NW):
                r0 = b * K + g * NW * P
                kt = kpool.tile([P, NW * D], f32, name="kt", tag="kt")
                src = keys_in[r0 : r0 + NW * P, :].rearrange(
                    "(c p) d -> p c d", p=P
                )
                nc.sync.dma_start(kt[:].rearrange("p (c d) -> p c d", c=NW), src)
                for c in range(NW):
                    t = g * NW + c
                    sl = kt[:, c * D : (c + 1) * D]
                    sq = spool.tile([P, D], f32, name="sq", tag="sq")
                    nc.scalar.activation(
                        sq[:],
                        sl,
                        mybir.ActivationFunctionType.Square,
                        accum_out=n2[:, t : t + 1],
                    )
                    prod = spool.tile([P, D], f32, name="prod", tag="prod")
                    nc.vector.scalar_tensor_tensor(
                        out=prod[:],
                        in0=sl,
                        scalar=1.0,
                        in1=wt[:],
                        op0=mybir.AluOpType.mult,
                        op1=mybir.AluOpType.mult,
                        accum_out=dot[:, t : t + 1],
                    )
                groups.append((kt, r0))

            if "g" in variant[2:]:
                # per-group inv: scales/stores of group g don't wait for
                # later groups' reductions
                for g, (kt, r0) in enumerate(groups):
                    cs = slice(g * NW, (g + 1) * NW)
                    sg = cpool.tile([P, NW], f32, name="sg", tag="sg")
                    nc.scalar.activation(
                        sg[:], n2[:, cs], mybir.ActivationFunctionType.Sqrt
                    )
                    nc.vector.tensor_scalar_add(sg[:], sg[:], EPS)
                    inv = cpool.tile([P, NW], f32, name="inv", tag="inv")
                    nc.vector.reciprocal(inv[:], sg[:])
                    nc.vector.tensor_mul(
                        sim_sb[:, b * TPB + g * NW : b * TPB + (g + 1) * NW],
                        dot[:, cs],
                        inv[:],
                    )
                    for c in range(NW):
                        sl = kt[:, c * D : (c + 1) * D]
                        nc.vector.tensor_scalar_mul(sl, sl, inv[:, c : c + 1])
                    dst = keys_out[r0 : r0 + NW * P, :].rearrange(
                        "(c p) d -> p c d", p=P
                    )
                    store_engine.dma_start(
                        dst, kt[:].rearrange("p (c d) -> p c d", c=NW)
                    )
            else:
                sg = cpool.tile([P, TPB], f32, name="sg", tag="sg")
                nc.scalar.activation(
                    sg[:], n2[:], mybir.ActivationFunctionType.Sqrt
                )
                nc.vector.tensor_scalar_add(sg[:], sg[:], EPS)
                inv = cpool.tile([P, TPB], f32, name="inv", tag="inv")
                nc.vector.reciprocal(inv[:], sg[:])
                nc.vector.tensor_mul(
                    sim_sb[:, b * TPB : (b + 1) * TPB], dot[:], inv[:]
                )

                for g, (kt, r0) in enumerate(groups):
                    for c in range(NW):
                        t = g * NW + c
                        sl = kt[:, c * D : (c + 1) * D]
                        nc.vector.tensor_scalar_mul(sl, sl, inv[:, t : t + 1])
                    dst = keys_out[r0 : r0 + NW * P, :].rearrange(
                        "(c p) d -> p c d", p=P
                    )
                    store_engine.dma_start(
                        dst, kt[:].rearrange("p (c d) -> p c d", c=NW)
                    )

        nc.sync.dma_start(sim_out[:], sim_sb[:])


def _build_program(reps=1, variant="v1"):
    import concourse.bacc as bacc
    import concourse.mybir as mybir
    import concourse.tile as tile

    f32 = mybir.dt.float32
    nc = bacc.Bacc(
        "TRN2",
        target_bir_lowering=False,
        debug=False,
        num_devices=M,
    )

    keys_in = nc.dram_tensor("keys_in", [BPC * K, D], f32, kind="ExternalInput").ap()
    vals_in = nc.dram_tensor("vals_in", [BPC * K, D], f32, kind="ExternalInput").ap()
    if variant == "v2b":
        wsrc = nc.dram_tensor("wkn", [BPC, D], f32, kind="ExternalInput").ap()
    else:
        wsrc = nc.dram_tensor("wbc", [BPC, P, D], f32, kind="ExternalInput").ap()
    keys_out = nc.dram_tensor("keys_out", [BPC * K, D], f32, kind="ExternalOutput").ap()
    vals_out = nc.dram_tensor("vals_out", [BPC * K, D], f32, kind="ExternalOutput").ap()
    sim_out = nc.dram_tensor("sim_out", [P, BPC * TPB], f32, kind="ExternalOutput").ap()

    with tile.TileContext(nc) as tc:
        for _rep in range(reps):
            _emit_body(nc, tc, mybir, f32, keys_in, vals_in, wsrc,
                       keys_out, vals_out, sim_out, variant=variant)

    nc.compile()
    return nc


DEFAULT_VARIANT = "v4g"


def _get_program():
    global _PROGRAM
    if _PROGRAM is None:
        _PROGRAM = _build_program(
            variant=os.environ.get("KERNEL_VARIANT", DEFAULT_VARIANT)
        )
    return _PROGRAM


def kernel(**inputs):
    global LAST_RESULT
    from concourse.bass_utils import run_bass_kernel_spmd

    wk = np.asarray(inputs["write_key"], dtype=np.float32)
    wv = np.asarray(inputs["write_val"], dtype=np.float32)
    ws_raw = np.asarray(inputs["write_strength"], dtype=np.float32)
    ek = np.ascontiguousarray(np.asarray(inputs["epi_keys"], dtype=np.float32))
    ev = np.ascontiguousarray(np.asarray(inputs["epi_vals"], dtype=np.float32))
    ea = np.asarray(inputs["epi_age"], dtype=np.float32)
    es = np.asarray(inputs["epi_strength"], dtype=np.float32)

    nc = _get_program()

    wk_n = wk / (np.linalg.norm(wk, axis=-1, keepdims=True) + np.float32(EPS))
    in_maps = []
    for c in range(M):
        sl = slice(c * BPC, (c + 1) * BPC)
        in_maps.append(
            {
                "keys_in": ek[sl].reshape(BPC * K, D),
                "vals_in": ev[sl].reshape(BPC * K, D),
                "wbc": np.ascontiguousarray(
                    np.broadcast_to(wk_n[sl][:, None, :], (BPC, P, D))
                ),
            }
        )

    trace = bool(int(os.environ.get("KERNEL_TRACE", "0")))
    try:
        res = run_bass_kernel_spmd(nc, in_maps, list(range(M)), trace=trace)
    except Exception:
        # one retry for transient runtime errors (e.g. a device left wedged
        # by a previous tenant's crashed session)
        import time as _time

        _time.sleep(5)
        res = run_bass_kernel_spmd(nc, in_maps, list(range(M)), trace=trace)
    LAST_RESULT = res

    keys_new = np.empty((B, K, D), dtype=np.float32)
    vals_new = np.empty((B, K, D), dtype=np.float32)
    sim = np.empty((B, K), dtype=np.float32)
    for c in range(M):
        sl = slice(c * BPC, (c + 1) * BPC)
        keys_new[sl] = res.results[c]["keys_out"].reshape(BPC, K, D)
        vals_new[sl] = res.results[c]["vals_out"].reshape(BPC, K, D)
        # sim_out is [p, b*TPB + t] with k = t*P + p
        sim_raw = res.results[c]["sim_out"].reshape(P, BPC, TPB)
        sim[sl] = sim_raw.transpose(1, 2, 0).reshape(BPC, K)

    top = sim.argmax(axis=1)
    best_sim = sim.max(axis=1)
    slot_idx = top.astype(np.int32)

    ws = np.clip(ws_raw, 0.0, 1.0)
    r = ws * np.float32(0.5)  # (B,)
    ar = np.arange(B)

    # fix the written row of keys (blend then normalize)
    kb = (1.0 - r)[:, None] * ek[ar, top] + r[:, None] * wk
    kbn = kb / (np.linalg.norm(kb, axis=-1, keepdims=True) + np.float32(EPS))
    keys_new[ar, top] = kbn

    # fix the written row of vals
    vals_new[ar, top] = (1.0 - r)[:, None] * ev[ar, top] + r[:, None] * wv

    age_new = ea + np.float32(1.0)
    age_new[ar, top] = 0.0

    s0 = es * np.float32(0.995)
    str_new = np.clip(s0, 0.001, 1.0).astype(np.float32)
    str_new[ar, top] = np.clip(
        s0[ar, top] + ws * (np.float32(1.0) - s0[ar, top]), 0.001, 1.0
    )

    return keys_new, vals_new, age_new, str_new, slot_idx, best_sim


# revision 41
# speedup vs baseline: 1.4772x; 1.0255x over previous
"""EpisodicSlotWriter Trainium2 kernel.

Math (forward values only — the straight-through trick makes write_w equal the
hard one-hot of argmax(sim)):
  sim[b,k]   = dot(epi_keys[b,k], wk_n[b]) / (||epi_keys[b,k]|| + EPS)
  top[b]     = argmax_k sim[b,k];  best[b] = max_k sim[b,k]
  keys_new   = normalize(epi_keys) rows, except row top[b] which is the blended
               row (1-r)*epi_keys + r*write_key, normalized; r = 0.5*clip(ws,0,1)
  vals_new   = epi_vals, except row top[b] blended the same way
  age_new    = (epi_age+1) zeroed at top[b]
  str_new    = clip(0.995*str [+ ws*(1-0.995*str) at top[b]], 0.001, 1.0)

Device (8 NeuronCores, batch-parallel, 8 batches/core) streams the heavy
tensors: normalizes every epi_keys row + computes sim (the full 256MB keys
in/out), and copies epi_vals to vals_new (256MB in/out) via DRAM->DRAM DMA.
Host does the O(B*D) fix-up of the single written row per batch plus the tiny
(B,K) age/strength maps.
"""

import os
from contextlib import ExitStack

import numpy as np

EPS = 1e-6
B, K, D = 64, 2048, 512
M = 8            # cores
BPC = B // M     # batches per core
P = 128          # partitions
TPB = K // P     # k-tiles per batch

_PROGRAM = None
LAST_RESULT = None


def _emit_body(nc, tc, mybir, f32, keys_in, vals_in, wsrc,
               keys_out, vals_out, sim_out, variant="v1"):
    if variant == "vonly":
        # diagnostic: only the vals DRAM->DRAM copy
        n_chunks = 8
        rpc = BPC * K // n_chunks
        for c in range(n_chunks):
            nc.sync.dma_start(
                out=vals_out[c * rpc : (c + 1) * rpc, :],
                in_=vals_in[c * rpc : (c + 1) * rpc, :],
            )
        return
    if variant == "vsbuf":
        # diagnostic: vals copy routed through SBUF in [128, 2048] tiles
        NW = 4
        with ExitStack() as ctx:
            vpool = ctx.enter_context(tc.tile_pool(name="vpool", bufs=6))
            for g in range(BPC * K // (NW * P)):
                r0 = g * NW * P
                vt = vpool.tile([P, NW * D], f32, name="vt", tag="vt")
                src = vals_in[r0 : r0 + NW * P, :].rearrange(
                    "(c p) d -> p c d", p=P
                )
                dst = vals_out[r0 : r0 + NW * P, :].rearrange(
                    "(c p) d -> p c d", p=P
                )
                vt3 = vt[:].rearrange("p (c d) -> p c d", c=NW)
                nc.sync.dma_start(vt3, src)
                nc.sync.dma_start(dst, vt3)
        return
    if variant.startswith("v4") or variant.startswith("v8"):
        _emit_body_fused(nc, tc, mybir, f32, keys_in, vals_in, wsrc,
                         keys_out, vals_out, sim_out, variant)
        return
    with ExitStack() as ctx:
        kpool = ctx.enter_context(tc.tile_pool(name="kpool", bufs=24))
        spool = ctx.enter_context(tc.tile_pool(name="spool", bufs=3))
        wpool = ctx.enter_context(tc.tile_pool(name="wpool", bufs=2))
        cpool = ctx.enter_context(tc.tile_pool(name="cpool", bufs=3))
        simpool = ctx.enter_context(tc.tile_pool(name="simpool", bufs=1))
        if variant == "v2b":
            onepool = ctx.enter_context(tc.tile_pool(name="onepool", bufs=1))
            ppool = ctx.enter_context(
                tc.tile_pool(name="ppool", bufs=2, space="PSUM")
            )


        # Pass-through copy of epi_vals, DRAM->DRAM, in 4MB chunks.
        n_chunks = 8
        rpc = BPC * K // n_chunks
        for c in range(n_chunks):
            nc.sync.dma_start(
                out=vals_out[c * rpc : (c + 1) * rpc, :],
                in_=vals_in[c * rpc : (c + 1) * rpc, :],
            )

        sim_sb = simpool.tile([P, BPC * TPB], f32, name="sim_sb", tag="sim_sb")

        if variant == "v2b":
            ones = onepool.tile([1, P], f32, name="ones", tag="ones")
            nc.vector.memset(ones[:], 1.0)

        for b in range(BPC):
            if variant == "v2b":
                w1 = wpool.tile([1, D], f32, name="w1", tag="w1")
                nc.sync.dma_start(w1[:], wsrc[b : b + 1, :])
                wt = ppool.tile([P, D], f32, name="wp", tag="wp")
                nc.tensor.matmul(wt[:], lhsT=ones[:], rhs=w1[:],
                                 start=True, stop=True)
            else:
                wt = wpool.tile([P, D], f32, name="wt", tag="wt")
                nc.sync.dma_start(wt[:], wsrc[b])
            n2 = cpool.tile([P, TPB], f32, name="n2", tag="n2")
            dot = cpool.tile([P, TPB], f32, name="dot", tag="dot")
            kts = []
            for t in range(TPB):
                r0 = b * K + t * P
                kt = kpool.tile([P, D], f32, name="kt", tag="kt")
                nc.sync.dma_start(kt[:], keys_in[r0 : r0 + P, :])
                # ACT: norm2 per row (square + accumulate along free axis)
                sq = spool.tile([P, D], f32, name="sq", tag="sq")
                nc.scalar.activation(
                    sq[:],
                    kt[:],
                    mybir.ActivationFunctionType.Square,
                    accum_out=n2[:, t : t + 1],
                )
                # DVE: dot per row with the broadcast write-key
                # (tensor_tensor_reduce crashes the runtime; this
                # InstTensorScalarPtr form computes kt*wt with accum-sum)
                prod = spool.tile([P, D], f32, name="prod", tag="prod")
                nc.vector.scalar_tensor_tensor(
                    out=prod[:],
                    in0=kt[:],
                    scalar=1.0,
                    in1=wt[:],
                    op0=mybir.AluOpType.mult,
                    op1=mybir.AluOpType.mult,
                    accum_out=dot[:, t : t + 1],
                )
                kts.append((kt, r0))

            # inv = 1 / (sqrt(n2) + EPS), batched over the 16 tile columns
            sg = cpool.tile([P, TPB], f32, name="sg", tag="sg")
            nc.scalar.activation(sg[:], n2[:], mybir.ActivationFunctionType.Sqrt)
            nc.vector.tensor_scalar_add(sg[:], sg[:], EPS)
            inv = cpool.tile([P, TPB], f32, name="inv", tag="inv")
            nc.vector.reciprocal(inv[:], sg[:])
            nc.vector.tensor_mul(
                sim_sb[:, b * TPB : (b + 1) * TPB], dot[:], inv[:]
            )

            for t, (kt, r0) in enumerate(kts):
                nc.vector.tensor_scalar_mul(kt[:], kt[:], inv[:, t : t + 1])
                nc.sync.dma_start(keys_out[r0 : r0 + P, :], kt[:])

        nc.sync.dma_start(sim_out[:], sim_sb[:])


def _emit_body_fused(nc, tc, mybir, f32, keys_in, vals_in, wbc,
                     keys_out, vals_out, sim_out, variant):
    """Like the v1 body, but keys move in [128, NW*512] tiles: one 1MB DMA
    per NW k-chunks, compute on 512-wide slices. 'v8' additionally issues
    keys stores on the ACT HWDGE ring to split descriptor generation."""
    novals = "novals" in variant
    vlast = "vlast" in variant
    vkey = variant.replace("novals", "").replace("vlast", "")
    NW = 8 if "w8" in vkey else 4
    spread_vals = (not novals) and "s" in vkey[2:].replace("w8", "").replace(
        "b", ""
    )
    store_engine = nc.scalar if vkey.startswith("v8") else nc.sync
    nbufs = (8 if NW == 4 else 4)
    if "b" in vkey[2:]:
        nbufs = int(vkey.split("b")[1])
    variant = vkey
    with ExitStack() as ctx:
        kpool = ctx.enter_context(tc.tile_pool(name="kpool", bufs=nbufs))
        spool = ctx.enter_context(tc.tile_pool(name="spool", bufs=3))
        wpool = ctx.enter_context(tc.tile_pool(name="wpool", bufs=2))
        cpool = ctx.enter_context(tc.tile_pool(name="cpool", bufs=3))
        simpool = ctx.enter_context(tc.tile_pool(name="simpool", bufs=1))

        n_chunks = 8
        rpc = BPC * K // n_chunks
        if not spread_vals and not novals and not vlast:
            for c in range(n_chunks):
                nc.sync.dma_start(
                    out=vals_out[c * rpc : (c + 1) * rpc, :],
                    in_=vals_in[c * rpc : (c + 1) * rpc, :],
                )

        sim_sb = simpool.tile([P, BPC * TPB], f32, name="sim_sb", tag="sim_sb")

        for b in range(BPC):
            if spread_vals:
                nc.sync.dma_start(
                    out=vals_out[b * rpc : (b + 1) * rpc, :],
                    in_=vals_in[b * rpc : (b + 1) * rpc, :],
                )
            wt = wpool.tile([P, D], f32, name="wt", tag="wt")
            nc.sync.dma_start(wt[:], wbc[b])
            n2 = cpool.tile([P, TPB], f32, name="n2", tag="n2")
            dot = cpool.tile([P, TPB], f32, name="dot", tag="dot")
            groups = []
            for g in range(TPB // NW):
                r0 = b * K + g * NW * P
                kt = kpool.tile([P, NW * D], f32, name="kt", tag="kt")
                src = keys_in[r0 : r0 + NW * P, :].rearrange(
                    "(c p) d -> p c d", p=P
                )
                nc.sync.dma_start(kt[:].rearrange("p (c d) -> p c d", c=NW), src)
                for c in range(NW):
                    t = g * NW + c
                    sl = kt[:, c * D : (c + 1) * D]
                    sq = spool.tile([P, D], f32, name="sq", tag="sq")
                    nc.scalar.activation(
                        sq[:],
                        sl,
                        mybir.ActivationFunctionType.Square,
                        accum_out=n2[:, t : t + 1],
                    )
                    prod = spool.tile([P, D], f32, name="prod", tag="prod")
                    nc.vector.scalar_tensor_tensor(
                        out=prod[:],
                        in0=sl,
                        scalar=1.0,
                        in1=wt[:],
                        op0=mybir.AluOpType.mult,
                        op1=mybir.AluOpType.mult,
                        accum_out=dot[:, t : t + 1],
                    )
                groups.append((kt, r0))

            if "g" in variant[2:]:
                # per-group inv: scales/stores of group g don't wait for
                # later groups' reductions
                for g, (kt, r0) in enumerate(groups):
                    cs = slice(g * NW, (g + 1) * NW)
                    sg = cpool.tile([P, NW], f32, name="sg", tag="sg")
                    nc.scalar.activation(
                        sg[:], n2[:, cs], mybir.ActivationFunctionType.Sqrt
                    )
                    nc.vector.tensor_scalar_add(sg[:], sg[:], EPS)
                    inv = cpool.tile([P, NW], f32, name="inv", tag="inv")
                    nc.vector.reciprocal(inv[:], sg[:])
                    nc.vector.tensor_mul(
                        sim_sb[:, b * TPB + g * NW : b * TPB + (g + 1) * NW],
                        dot[:, cs],
                        inv[:],
                    )
                    for c in range(NW):
                        sl = kt[:, c * D : (c + 1) * D]
                        nc.vector.tensor_scalar_mul(sl, sl, inv[:, c : c + 1])
                    dst = keys_out[r0 : r0 + NW * P, :].rearrange(
                        "(c p) d -> p c d", p=P
                    )
                    store_engine.dma_start(
                        dst, kt[:].rearrange("p (c d) -> p c d", c=NW)
                    )
            else:
                sg = cpool.tile([P, TPB], f32, name="sg", tag="sg")
                nc.scalar.activation(
                    sg[:], n2[:], mybir.ActivationFunctionType.Sqrt
                )
                nc.vector.tensor_scalar_add(sg[:], sg[:], EPS)
                inv = cpool.tile([P, TPB], f32, name="inv", tag="inv")
                nc.vector.reciprocal(inv[:], sg[:])
                nc.vector.tensor_mul(
                    sim_sb[:, b * TPB : (b + 1) * TPB], dot[:], inv[:]
                )

                for g, (kt, r0) in enumerate(groups):
                    for c in range(NW):
                        t = g * NW + c
                        sl = kt[:, c * D : (c + 1) * D]
                        nc.vector.tensor_scalar_mul(sl, sl, inv[:, t : t + 1])
                    dst = keys_out[r0 : r0 + NW * P, :].rearrange(
                        "(c p) d -> p c d", p=P
                    )
                    store_engine.dma_start(
                        dst, kt[:].rearrange("p (c d) -> p c d", c=NW)
                    )

        if vlast:
            # emit the vals copy last: it lands at the tail of the DMA
            # lanes, temporally separating it from the keys stream (the
            # concurrent mix measures ~30% super-additive on HW)
            for c in range(n_chunks):
                nc.sync.dma_start(
                    out=vals_out[c * rpc : (c + 1) * rpc, :],
                    in_=vals_in[c * rpc : (c + 1) * rpc, :],
                )
        nc.sync.dma_start(sim_out[:], sim_sb[:])


def _build_program(reps=1, variant="v1"):
    import concourse.bacc as bacc
    import concourse.mybir as mybir
    import concourse.tile as tile

    f32 = mybir.dt.float32
    nc = bacc.Bacc(
        "TRN2",
        target_bir_lowering=False,
        debug=False,
        num_devices=M,
    )

    keys_in = nc.dram_tensor("keys_in", [BPC * K, D], f32, kind="ExternalInput").ap()
    vals_in = nc.dram_tensor("vals_in", [BPC * K, D], f32, kind="ExternalInput").ap()
    if variant == "v2b":
        wsrc = nc.dram_tensor("wkn", [BPC, D], f32, kind="ExternalInput").ap()
    else:
        wsrc = nc.dram_tensor("wbc", [BPC, P, D], f32, kind="ExternalInput").ap()
    keys_out = nc.dram_tensor("keys_out", [BPC * K, D], f32, kind="ExternalOutput").ap()
    vals_out = nc.dram_tensor("vals_out", [BPC * K, D], f32, kind="ExternalOutput").ap()
    sim_out = nc.dram_tensor("sim_out", [P, BPC * TPB], f32, kind="ExternalOutput").ap()

    with tile.TileContext(nc) as tc:
        for _rep in range(reps):
            _emit_body(nc, tc, mybir, f32, keys_in, vals_in, wsrc,
                       keys_out, vals_out, sim_out, variant=variant)

    nc.compile()
    return nc


DEFAULT_VARIANT = "v4gvlast"


def _get_program():
    global _PROGRAM
    if _PROGRAM is None:
        _PROGRAM = _build_program(
            variant=os.environ.get("KERNEL_VARIANT", DEFAULT_VARIANT)
        )
    return _PROGRAM


def kernel(**inputs):
    global LAST_RESULT
    from concourse.bass_utils import run_bass_kernel_spmd

    wk = np.asarray(inputs["write_key"], dtype=np.float32)
    wv = np.asarray(inputs["write_val"], dtype=np.float32)
    ws_raw = np.asarray(inputs["write_strength"], dtype=np.float32)
    ek = np.ascontiguousarray(np.asarray(inputs["epi_keys"], dtype=np.float32))
    ev = np.ascontiguousarray(np.asarray(inputs["epi_vals"], dtype=np.float32))
    ea = np.asarray(inputs["epi_age"], dtype=np.float32)
    es = np.asarray(inputs["epi_strength"], dtype=np.float32)

    nc = _get_program()

    wk_n = wk / (np.linalg.norm(wk, axis=-1, keepdims=True) + np.float32(EPS))
    in_maps = []
    for c in range(M):
        sl = slice(c * BPC, (c + 1) * BPC)
        in_maps.append(
            {
                "keys_in": ek[sl].reshape(BPC * K, D),
                "vals_in": ev[sl].reshape(BPC * K, D),
                "wbc": np.ascontiguousarray(
                    np.broadcast_to(wk_n[sl][:, None, :], (BPC, P, D))
                ),
            }
        )

    trace = bool(int(os.environ.get("KERNEL_TRACE", "0")))
    try:
        res = run_bass_kernel_spmd(nc, in_maps, list(range(M)), trace=trace)
    except Exception:
        # one retry for transient runtime errors (e.g. a device left wedged
        # by a previous tenant's crashed session)
        import time as _time

        _time.sleep(5)
        res = run_bass_kernel_spmd(nc, in_maps, list(range(M)), trace=trace)
    LAST_RESULT = res

    keys_new = np.empty((B, K, D), dtype=np.float32)
    vals_new = np.empty((B, K, D), dtype=np.float32)
    sim = np.empty((B, K), dtype=np.float32)
    for c in range(M):
        sl = slice(c * BPC, (c + 1) * BPC)
        keys_new[sl] = res.results[c]["keys_out"].reshape(BPC, K, D)
        vals_new[sl] = res.results[c]["vals_out"].reshape(BPC, K, D)
        # sim_out is [p, b*TPB + t] with k = t*P + p
        sim_raw = res.results[c]["sim_out"].reshape(P, BPC, TPB)
        sim[sl] = sim_raw.transpose(1, 2, 0).reshape(BPC, K)

    top = sim.argmax(axis=1)
    best_sim = sim.max(axis=1)
    slot_idx = top.astype(np.int32)

    ws = np.clip(ws_raw, 0.0, 1.0)
    r = ws * np.float32(0.5)  # (B,)
    ar = np.arange(B)

    # fix the written row of keys (blend then normalize)
    kb = (1.0 - r)[:, None] * ek[ar, top] + r[:, None] * wk
    kbn = kb / (np.linalg.norm(kb, axis=-1, keepdims=True) + np.float32(EPS))
    keys_new[ar, top] = kbn

    # fix the written row of vals
    vals_new[ar, top] = (1.0 - r)[:, None] * ev[ar, top] + r[:, None] * wv

    age_new = ea + np.float32(1.0)
    age_new[ar, top] = 0.0

    s0 = es * np.float32(0.995)
    str_new = np.clip(s0, 0.001, 1.0).astype(np.float32)
    str_new[ar, top] = np.clip(
        s0[ar, top] + ws * (np.float32(1.0) - s0[ar, top]), 0.001, 1.0
    )

    return keys_new, vals_new, age_new, str_new, slot_idx, best_sim
